# revision 13
# baseline (speedup 1.0000x reference)
"""Trainium2 Bass kernel for the AttentionLayer problem.

Computation (per batch b):
    f      = relu(W_lf @ features[b] + b_lf)         [H, L]
    fe     = W_ef @ f + b_ef                         [H, L]
    h      = tanh(W_lh @ hidden[b] + b_lh)           [H]
    he     = W_eh @ h + b_eh                         [H]
    merged = tanh(fe + he[:, None])                  [H, L]
    scores = w_att @ merged (+ b_att)                [L]
    att    = softmax(scores)                         [L]   (b_att cancels)
    v      = f @ att                                 [H]
    out    = tanh(W_out @ (v + h) + b_out)           [H]
Returns (out, att).

Strategy: data-parallel over batch B=32 across 8 NeuronCores (4 batches
per core), no collectives.  All GEMMs run on the tensor engine in bf16
with fp32 PSUM accumulation.  Weights are pre-transposed/cast on the
host so every matmul's stationary operand is a natural [K=128, M]
slice.  The v = f @ att contraction (over L, the free dim of f's
layout) runs on the vector engine as a fused multiply+reduce against a
PE-broadcast copy of att.
"""

import sys

sys.path.insert(0, "/opt/trn_rl_repo")

import numpy as np
import ml_dtypes

BF = ml_dtypes.bfloat16

B, E, H, L = 32, 1024, 1024, 2048
NCORES = 8
BC = B // NCORES  # batches per core
KT = E // 128     # contraction tiles (E == H)
MT = H // 128     # output-row tiles

_prog_cache = {}

# pool buffer tunables
PMM_BUFS = 3
PSMALL_BUFS = 2
FPOOL_BUFS = 2
FEATP_BUFS = 2
MPOOL_BUFS = 1
APOOL_BUFS = 2
SPOOL_BUFS = 1
ROWS_BUFS = 1
VPOOL_BUFS = 2
DEBUG_SKIP = set()  # build-time stage stubs for timeline attribution


def _build_program():
    """Build the single-core Bass program (identical on all 8 cores)."""
    if "nc" in _prog_cache:
        return _prog_cache["nc"]

    from contextlib import ExitStack

    import concourse.bass as bass
    import concourse.tile as tile
    from concourse import bacc, mybir

    fp32 = mybir.dt.float32
    bf16 = mybir.dt.bfloat16
    AF = mybir.ActivationFunctionType
    ALU = mybir.AluOpType
    PSUM = bass.MemorySpace.PSUM

    nc = bacc.Bacc("TRN2", target_bir_lowering=False, debug=False)

    def din(name, shape, dtype=bf16):
        return nc.declare_dram_parameter(name, list(shape), dtype, isOutput=False).ap()

    # [b, cp, p, k, n] = features[b, k*128+p, cp*1024+n], bf16
    feats_d = din("feats", [BC, 2, 128, KT, 1024])
    w_lfT_d = din("w_lfT", [128, KT, H])
    w_efT_d = din("w_efT", [128, KT, H])
    w_lhT_d = din("w_lhT", [128, KT, H])
    w_ehT_d = din("w_ehT", [128, KT, H])
    w_outT_d = din("w_outT", [128, KT, H])
    watt_d = din("watt", [128, KT])
    hidT_d = din("hidT", [128, KT, BC])
    blf_d = din("blf", [128, MT], fp32)
    bef_d = din("bef", [128, MT], fp32)
    blh_d = din("blh", [128, MT], fp32)
    beh_d = din("beh", [128, MT], fp32)
    bout_d = din("bout", [BC, H], fp32)

    out_d = nc.declare_dram_parameter("out", [BC, H], fp32, isOutput=True).ap()
    att_d = nc.declare_dram_parameter("att", [BC, L], fp32, isOutput=True).ap()

    with ExitStack() as ctx:
        tc = ctx.enter_context(tile.TileContext(nc))

        # PSUM: pmm 3x[128,1024] (6 banks) + psmall 2x[<=128,512] (2 banks)
        pmm = ctx.enter_context(tc.tile_pool(name="pmm", bufs=PMM_BUFS, space=PSUM))
        psmall = ctx.enter_context(
            tc.tile_pool(name="psmall", bufs=PSMALL_BUFS, space=PSUM))

        consts = ctx.enter_context(tc.tile_pool(name="consts", bufs=1))

        def cload(dram_ap, shape, dtype, tag):
            t = consts.tile(list(shape), dtype, tag=tag)
            nc.sync.dma_start(out=t, in_=dram_ap)
            return t

        w_lfT = cload(w_lfT_d, [128, KT, H], bf16, "w_lfT")
        w_efT = cload(w_efT_d, [128, KT, H], bf16, "w_efT")
        w_outT = cload(w_outT_d, [128, KT, H], bf16, "w_outT")
        watt = cload(watt_d, [128, KT], bf16, "watt")
        blf = cload(blf_d, [128, MT], fp32, "blf")
        bef = cload(bef_d, [128, MT], fp32, "bef")
        blh = cload(blh_d, [128, MT], fp32, "blh")
        beh = cload(beh_d, [128, MT], fp32, "beh")
        outbias = cload(bout_d, [BC, H], fp32, "outbias")

        ones_f = consts.tile([1, 128], fp32, tag="ones")
        nc.vector.memset(ones_f[:, :], 1.0)

        h_f = consts.tile([128, MT, BC], fp32, tag="h_f")
        h_bf = consts.tile([128, MT, BC], bf16, tag="h_bf")
        he_f = consts.tile([128, MT, BC], fp32, tag="he_f")
        vh_bf = consts.tile([128, MT, BC], bf16, tag="vh_bf")

        # ---- hidden path: h = tanh(W_lh @ hidden + b_lh); he = W_eh @ h + b_eh
        with tc.tile_pool(name="hpool", bufs=1) as hpool:
            w_lhT = hpool.tile([128, KT, H], bf16, tag="w_lhT")
            nc.sync.dma_start(out=w_lhT, in_=w_lhT_d)
            w_ehT = hpool.tile([128, KT, H], bf16, tag="w_ehT")
            nc.sync.dma_start(out=w_ehT, in_=w_ehT_d)
            hidT = hpool.tile([128, KT, BC], bf16, tag="hidT")
            nc.sync.dma_start(out=hidT, in_=hidT_d)

            for m in range(MT):
                ps = psmall.tile([128, 512], fp32, tag="small")
                for k in range(KT):
                    nc.tensor.matmul(
                        ps[:, :BC],
                        w_lhT[:, k, m * 128:(m + 1) * 128],
                        hidT[:, k, :],
                        start=(k == 0),
                        stop=(k == KT - 1),
                    )
                nc.scalar.activation(h_f[:, m, :], ps[:, :BC], AF.Tanh,
                                     bias=blh[:, m:m + 1])
            nc.vector.tensor_copy(h_bf[:, :, :], h_f[:, :, :])
            for m in range(MT):
                ps = psmall.tile([128, 512], fp32, tag="small")
                for k in range(KT):
                    nc.tensor.matmul(
                        ps[:, :BC],
                        w_ehT[:, k, m * 128:(m + 1) * 128],
                        h_bf[:, k, :],
                        start=(k == 0),
                        stop=(k == KT - 1),
                    )
                nc.scalar.activation(he_f[:, m, :], ps[:, :BC], AF.Identity,
                                     bias=beh[:, m:m + 1])

        # ---- main pools
        fpool = ctx.enter_context(tc.tile_pool(name="fpool", bufs=FPOOL_BUFS))
        featp = ctx.enter_context(tc.tile_pool(name="featp", bufs=FEATP_BUFS))
        mpool = ctx.enter_context(tc.tile_pool(name="mpool", bufs=MPOOL_BUFS))
        apool = ctx.enter_context(tc.tile_pool(name="apool", bufs=APOOL_BUFS))
        spool = ctx.enter_context(tc.tile_pool(name="spool", bufs=SPOOL_BUFS))
        rows = ctx.enter_context(tc.tile_pool(name="rows", bufs=ROWS_BUFS))
        vpool = ctx.enter_context(tc.tile_pool(name="vpool", bufs=VPOOL_BUFS))

        for b in range(BC):
            f_sb = fpool.tile([128, KT, L], bf16, tag="f")
            scores = rows.tile([1, L], fp32, tag="scores")

            for cp in range(2):
                ft = featp.tile([128, KT, 1024], bf16, tag="ft")
                nc.sync.dma_start(out=ft, in_=feats_d[b, cp])

                # f = relu(W_lf @ features + b_lf), cast to bf16
                for m in range(MT):
                    ps = pmm.tile([128, 1024], fp32, tag="mm")
                    for h2 in range(2):
                        sl = slice(h2 * 512, (h2 + 1) * 512)
                        for k in range(KT):
                            nc.tensor.matmul(
                                ps[:, sl],
                                w_lfT[:, k, m * 128:(m + 1) * 128],
                                ft[:, k, sl],
                                start=(k == 0),
                                stop=(k == KT - 1),
                            )
                    nc.scalar.activation(
                        f_sb[:, m, cp * 1024:(cp + 1) * 1024], ps, AF.Relu,
                        bias=blf[:, m:m + 1])

                # merged = tanh(W_ef @ f + b_ef + he)
                merged = mpool.tile([128, MT, 1024], bf16, tag="m")
                for m in range(MT):
                    ps = pmm.tile([128, 1024], fp32, tag="mm")
                    for h2 in range(2):
                        sl = slice(h2 * 512, (h2 + 1) * 512)
                        for k in range(KT):
                            nc.tensor.matmul(
                                ps[:, sl],
                                w_efT[:, k, m * 128:(m + 1) * 128],
                                f_sb[:, k, cp * 1024 + h2 * 512:
                                     cp * 1024 + (h2 + 1) * 512],
                                start=(k == 0),
                                stop=(k == KT - 1),
                            )
                    # bias = b_ef + he (precombined on host is not possible:
                    # he is computed on-device; add b_ef into he once instead)
                    nc.scalar.activation(merged[:, m, :], ps, AF.Tanh,
                                         bias=he_f[:, m, b:b + 1])

                # scores = w_att @ merged
                for h2 in range(2):
                    ps = psmall.tile([128, 512], fp32, tag="small")
                    for k in range(KT):
                        nc.tensor.matmul(
                            ps[:1, :],
                            watt[:, k:k + 1],
                            merged[:, k, h2 * 512:(h2 + 1) * 512],
                            start=(k == 0),
                            stop=(k == KT - 1),
                        )
                    nc.vector.tensor_copy(
                        scores[:, cp * 1024 + h2 * 512:cp * 1024 + (h2 + 1) * 512],
                        ps[:1, :])

            # ---- softmax over L (single partition row)
            srow = rows.tile([1, 4], fp32, tag="srow")
            attrow = rows.tile([1, L], fp32, tag="attrow")
            if "softmax" in DEBUG_SKIP:
                nc.vector.tensor_copy(attrow, scores)
            else:
                nc.vector.tensor_reduce(srow[:, 0:1], scores,
                                        axis=mybir.AxisListType.X, op=ALU.max)
                nc.scalar.mul(srow[:, 1:2], srow[:, 0:1], -1.0)
                nc.scalar.activation(attrow, scores, AF.Exp, bias=srow[:, 1:2],
                                     scale=1.0, accum_out=srow[:, 2:3])
                nc.vector.reciprocal(srow[:, 3:4], srow[:, 2:3])
                nc.vector.tensor_scalar_mul(attrow, attrow, srow[:, 3:4])
            nc.sync.dma_start(out=att_d[b:b + 1, :], in_=attrow)

            # broadcast att across 128 partitions via PE outer product (fp32)
            attbc = apool.tile([128, L], bf16, tag="attbc")
            for c in range(4):
                ps = psmall.tile([128, 512], fp32, tag="small")
                nc.tensor.matmul(ps, ones_f, attrow[:, c * 512:(c + 1) * 512],
                                 start=True, stop=True)
                nc.vector.tensor_copy(attbc[:, c * 512:(c + 1) * 512], ps)

            # v[h] = sum_l f[h, l] * att[l]  (multiply + reduce on DVE;
            # InstTensorTensorReduce is not supported by this runtime)
            vt = vpool.tile([128, MT], fp32, tag="v")
            scr = spool.tile([128, L], bf16, tag="scr")
            if "v" in DEBUG_SKIP:
                nc.vector.memset(vt[:, :], 0.0)
            else:
                for k in range(MT):
                    nc.vector.tensor_mul(scr, f_sb[:, k, :], attbc)
                    nc.vector.tensor_reduce(vt[:, k:k + 1], scr,
                                            axis=mybir.AxisListType.X, op=ALU.add)
            # vh = v + h (bf16 for the final matmul)
            nc.vector.tensor_add(vh_bf[:, :, b], vt[:, :], h_f[:, :, b])

        # ---- out = tanh(W_out @ vh + b_out), batches as the 4-row moving dim
        out_sb = consts.tile([BC, H], fp32, tag="out_sb")
        for h2 in range(2):
            sl = slice(h2 * 512, (h2 + 1) * 512)
            ps = psmall.tile([128, 512], fp32, tag="small")
            for k in range(KT):
                nc.tensor.matmul(
                    ps[:BC, :],
                    vh_bf[:, k, :],
                    w_outT[:, k, sl],
                    start=(k == 0),
                    stop=(k == KT - 1),
                )
            nc.vector.tensor_add(out_sb[:, sl], ps[:BC, :], outbias[:, sl])
            nc.scalar.activation(out_sb[:, sl], out_sb[:, sl], AF.Tanh)
        nc.sync.dma_start(out=out_d, in_=out_sb)

    nc.compile()
    _prog_cache["nc"] = nc
    return nc


def _prep_inputs(features, hidden, W_lf, b_lf, W_ef, b_ef, W_lh, b_lh,
                 W_eh, b_eh, w_att, b_att, W_out, b_out):
    """Host-side sharding + layout prep. Returns list of 8 per-core maps."""

    def wT(W):  # [H_out, H_in] -> [128, KT, H_out] with [p,k,j] = W[j, k*128+p]
        t = np.ascontiguousarray(
            W.T.astype(BF).reshape(KT, 128, H).transpose(1, 0, 2))
        return t

    def br(v):  # [H] -> [128, MT] with [p,m] = v[m*128+p]
        return np.ascontiguousarray(
            v.astype(np.float32).reshape(MT, 128).T)

    w_lfT = wT(W_lf)
    w_efT = wT(W_ef)
    w_lhT = wT(W_lh)
    w_ehT = wT(W_eh)
    w_outT = wT(W_out)
    watt = np.ascontiguousarray(w_att.astype(BF).reshape(KT, 128).T)
    blf = br(b_lf)
    # fold b_ef into the merged bias path? b_ef enters before tanh along with
    # he; he already gets +b_eh on device. Keep b_ef separate: it is added as
    # the ACT bias of the merged tanh together with he, so fold b_ef into he
    # by adding it to b_eh's bias... he and b_ef are both per-H constants
    # added in the same place: merged = tanh(fe_mm + b_ef + he). We pass
    # bias=he via ACT; so fold b_ef into b_eh (he = W_eh@h + b_eh + b_ef).
    beh_folded = br(b_eh + b_ef)
    bef = br(b_ef)  # unused on device; kept for layout stability
    blh = br(b_lh)
    bout = np.ascontiguousarray(
        np.broadcast_to(b_out.astype(np.float32), (BC, H)))

    feats_all = features.astype(BF)
    hid_all = hidden.astype(BF)

    in_maps = []
    for c in range(NCORES):
        fs = feats_all[c * BC:(c + 1) * BC]  # [BC, E, L]
        feats_r = np.ascontiguousarray(
            fs.reshape(BC, KT, 128, 2, 1024).transpose(0, 3, 2, 1, 4))
        hs = hid_all[c * BC:(c + 1) * BC]  # [BC, H]
        hidT = np.ascontiguousarray(
            hs.T.reshape(KT, 128, BC).transpose(1, 0, 2))
        in_maps.append({
            "feats": feats_r,
            "w_lfT": w_lfT,
            "w_efT": w_efT,
            "w_lhT": w_lhT,
            "w_ehT": w_ehT,
            "w_outT": w_outT,
            "watt": watt,
            "hidT": hidT,
            "blf": blf,
            "bef": bef,
            "blh": blh,
            "beh": beh_folded,
            "bout": bout,
        })
    return in_maps


def kernel(**inputs):
    inputs = {k: np.asarray(v) for k, v in inputs.items()}
    nc = _build_program()
    in_maps = _prep_inputs(**inputs)

    from concourse.bass_utils import run_bass_kernel_spmd

    bkr = run_bass_kernel_spmd(nc, in_maps, list(range(NCORES)))
    kernel.last_results = bkr

    out = np.concatenate([bkr.results[c]["out"] for c in range(NCORES)], axis=0)
    att = np.concatenate([bkr.results[c]["att"] for c in range(NCORES)], axis=0)
    return out.astype(np.float32), att.astype(np.float32)


# revision 20
# speedup vs baseline: 131.0817x; 131.0817x over previous
"""Trainium2 Bass kernel for the AttentionLayer problem.

Computation (per batch b):
    f      = relu(W_lf @ features[b] + b_lf)         [H, L]
    fe     = W_ef @ f + b_ef                         [H, L]
    h      = tanh(W_lh @ hidden[b] + b_lh)           [H]
    he     = W_eh @ h + b_eh                         [H]
    merged = tanh(fe + he[:, None])                  [H, L]
    scores = w_att @ merged (+ b_att)                [L]
    att    = softmax(scores)                         [L]   (b_att cancels)
    v      = f @ att                                 [H]
    out    = tanh(W_out @ (v + h) + b_out)           [H]
Returns (out, att).

Strategy: data-parallel over batch B=32 across 8 NeuronCores (4 batches
per core), no collectives.  All GEMMs run on the tensor engine in bf16
with fp32 PSUM accumulation.  Weights are pre-transposed/cast on the
host so every matmul's stationary operand is a natural [K=128, M]
slice.  The v = f @ att contraction (over L, the free dim of f's
layout) runs on the vector engine as a fused multiply+reduce against a
PE-broadcast copy of att.
"""

import sys

sys.path.insert(0, "/opt/trn_rl_repo")

import numpy as np
import ml_dtypes

BF = ml_dtypes.bfloat16

B, E, H, L = 32, 1024, 1024, 2048
NCORES = 8
BC = B // NCORES  # batches per core
KT = E // 128     # contraction tiles (E == H)
MT = H // 128     # output-row tiles

_prog_cache = {}

# pool buffer tunables
PMM_BUFS = 3
PSMALL_BUFS = 2
FPOOL_BUFS = 2
FEATP_BUFS = 2
MPOOL_BUFS = 1
APOOL_BUFS = 2
SPOOL_BUFS = 1
ROWS_BUFS = 1
VPOOL_BUFS = 2
DEBUG_SKIP = set()  # build-time stage stubs for timeline attribution


def _build_program():
    """Build the single-core Bass program (identical on all 8 cores)."""
    if "nc" in _prog_cache:
        return _prog_cache["nc"]

    from contextlib import ExitStack

    import concourse.bass as bass
    import concourse.tile as tile
    from concourse import bacc, mybir

    fp32 = mybir.dt.float32
    bf16 = mybir.dt.bfloat16
    AF = mybir.ActivationFunctionType
    ALU = mybir.AluOpType
    PSUM = bass.MemorySpace.PSUM

    nc = bacc.Bacc("TRN2", target_bir_lowering=False, debug=False)

    def din(name, shape, dtype=bf16):
        return nc.declare_dram_parameter(name, list(shape), dtype, isOutput=False).ap()

    # [b, cp, p, k, n] = features[b, k*128+p, cp*1024+n], bf16
    feats_d = din("feats", [BC, 2, 128, KT, 1024])
    w_lfT_d = din("w_lfT", [128, KT, H])
    w_efT_d = din("w_efT", [128, KT, H])
    w_lhT_d = din("w_lhT", [128, KT, H])
    w_ehT_d = din("w_ehT", [128, KT, H])
    w_outT_d = din("w_outT", [128, KT, H])
    watt_d = din("watt", [128, KT])
    hidT_d = din("hidT", [128, KT, BC])
    blf_d = din("blf", [128, MT], fp32)
    bef_d = din("bef", [128, MT], fp32)
    blh_d = din("blh", [128, MT], fp32)
    beh_d = din("beh", [128, MT], fp32)
    bout_d = din("bout", [BC, H], fp32)

    out_d = nc.declare_dram_parameter("out", [BC, H], fp32, isOutput=True).ap()
    att_d = nc.declare_dram_parameter("att", [BC, L], fp32, isOutput=True).ap()

    with ExitStack() as ctx:
        tc = ctx.enter_context(tile.TileContext(nc))

        # PSUM: pmm 3x[128,1024] (6 banks) + psmall 2x[<=128,512] (2 banks)
        pmm = ctx.enter_context(tc.tile_pool(name="pmm", bufs=PMM_BUFS, space=PSUM))
        psmall = ctx.enter_context(
            tc.tile_pool(name="psmall", bufs=PSMALL_BUFS, space=PSUM))

        consts = ctx.enter_context(tc.tile_pool(name="consts", bufs=1))

        def cload(dram_ap, shape, dtype, tag):
            t = consts.tile(list(shape), dtype, tag=tag)
            nc.sync.dma_start(out=t, in_=dram_ap)
            return t

        # small constants first on the sync DMA ring
        watt = cload(watt_d, [128, KT], bf16, "watt")
        blf = cload(blf_d, [128, MT], fp32, "blf")
        blh = cload(blh_d, [128, MT], fp32, "blh")
        beh = cload(beh_d, [128, MT], fp32, "beh")
        outbias = cload(bout_d, [BC, H], fp32, "outbias")
        hidT = cload(hidT_d, [128, KT, BC], bf16, "hidT")

        ones_f = consts.tile([1, 128], fp32, tag="ones")
        nc.vector.memset(ones_f[:, :], 1.0)

        h_f = consts.tile([128, MT, BC], fp32, tag="h_f")
        h_bf = consts.tile([128, MT, BC], bf16, tag="h_bf")
        he_f = consts.tile([128, MT, BC], fp32, tag="he_f")
        vh_bf = consts.tile([128, MT, BC], bf16, tag="vh_bf")

        # ---- hidden path first on the PE stream with its weights' DMAs
        # queued ahead of the big GEMM weights (transient pool, freed after)
        with tc.tile_pool(name="hpool", bufs=1) as hpool:
            w_lhT = hpool.tile([128, KT, H], bf16, tag="w_lhT")
            nc.sync.dma_start(out=w_lhT, in_=w_lhT_d)
            w_ehT = hpool.tile([128, KT, H], bf16, tag="w_ehT")
            nc.sync.dma_start(out=w_ehT, in_=w_ehT_d)
            w_lfT = cload(w_lfT_d, [128, KT, H], bf16, "w_lfT")
            w_efT = cload(w_efT_d, [128, KT, H], bf16, "w_efT")

            if "hidden" in DEBUG_SKIP:
                nc.vector.memset(h_f[:, :, :], 0.0)
                nc.vector.memset(h_bf[:, :, :], 0.0)
                nc.vector.memset(he_f[:, :, :], 0.0)
            for m in range(MT) if "hidden" not in DEBUG_SKIP else []:
                ps = psmall.tile([128, 512], fp32, tag="small")
                for k in range(KT):
                    nc.tensor.matmul(
                        ps[:, :BC],
                        w_lhT[:, k, m * 128:(m + 1) * 128],
                        hidT[:, k, :],
                        start=(k == 0),
                        stop=(k == KT - 1),
                    )
                nc.scalar.activation(h_f[:, m, :], ps[:, :BC], AF.Tanh,
                                     bias=blh[:, m:m + 1])
            if "hidden" not in DEBUG_SKIP:
                nc.vector.tensor_copy(h_bf[:, :, :], h_f[:, :, :])
            for m in range(MT) if "hidden" not in DEBUG_SKIP else []:
                ps = psmall.tile([128, 512], fp32, tag="small")
                for k in range(KT):
                    nc.tensor.matmul(
                        ps[:, :BC],
                        w_ehT[:, k, m * 128:(m + 1) * 128],
                        h_bf[:, k, :],
                        start=(k == 0),
                        stop=(k == KT - 1),
                    )
                nc.scalar.activation(he_f[:, m, :], ps[:, :BC], AF.Identity,
                                     bias=beh[:, m:m + 1])

        # ---- main pools
        fpool = ctx.enter_context(tc.tile_pool(name="fpool", bufs=FPOOL_BUFS))
        featp = ctx.enter_context(tc.tile_pool(name="featp", bufs=FEATP_BUFS))
        mpool = ctx.enter_context(tc.tile_pool(name="mpool", bufs=MPOOL_BUFS))
        apool = ctx.enter_context(tc.tile_pool(name="apool", bufs=APOOL_BUFS))
        spool = ctx.enter_context(tc.tile_pool(name="spool", bufs=SPOOL_BUFS))
        rows = ctx.enter_context(tc.tile_pool(name="rows", bufs=ROWS_BUFS))
        vpool = ctx.enter_context(tc.tile_pool(name="vpool", bufs=VPOOL_BUFS))

        w_outT = cload(w_outT_d, [128, KT, H], bf16, "w_outT")

        def emit_F(b, cp, f_sb, ft):
            # f = relu(W_lf @ features + b_lf), cast to bf16
            for m in range(MT):
                ps = pmm.tile([128, 1024], fp32, tag="mm")
                for h2 in range(2):
                    sl = slice(h2 * 512, (h2 + 1) * 512)
                    for k in range(KT):
                        nc.tensor.matmul(
                            ps[:, sl],
                            w_lfT[:, k, m * 128:(m + 1) * 128],
                            ft[:, k, sl],
                            start=(k == 0),
                            stop=(k == KT - 1),
                        )
                nc.scalar.activation(
                    f_sb[:, m, cp * 1024:(cp + 1) * 1024], ps, AF.Relu,
                    bias=blf[:, m:m + 1])

        def emit_FE_S(b, cp, f_sb, scores):
            # merged = tanh(W_ef @ f + (he + b_eh + b_ef))
            merged = mpool.tile([128, MT, 1024], bf16, tag="m")
            for m in range(MT):
                ps = pmm.tile([128, 1024], fp32, tag="mm")
                for h2 in range(2):
                    sl = slice(h2 * 512, (h2 + 1) * 512)
                    for k in range(KT):
                        nc.tensor.matmul(
                            ps[:, sl],
                            w_efT[:, k, m * 128:(m + 1) * 128],
                            f_sb[:, k, cp * 1024 + h2 * 512:
                                 cp * 1024 + (h2 + 1) * 512],
                            start=(k == 0),
                            stop=(k == KT - 1),
                        )
                nc.scalar.activation(merged[:, m, :], ps, AF.Tanh,
                                     bias=he_f[:, m, b:b + 1])
            # scores = w_att @ merged
            for h2 in range(2):
                ps = psmall.tile([128, 512], fp32, tag="small")
                for k in range(KT):
                    nc.tensor.matmul(
                        ps[:1, :],
                        watt[:, k:k + 1],
                        merged[:, k, h2 * 512:(h2 + 1) * 512],
                        start=(k == 0),
                        stop=(k == KT - 1),
                    )
                nc.vector.tensor_copy(
                    scores[:, cp * 1024 + h2 * 512:cp * 1024 + (h2 + 1) * 512],
                    ps[:1, :])

        for b in range(BC):
            f_sb = fpool.tile([128, KT, L], bf16, tag="f")
            scores = rows.tile([1, L], fp32, tag="scores")

            for cp in range(2):
                ft = featp.tile([128, KT, 1024], bf16, tag="ft")
                # scalar HWDGE ring: runs parallel to the sync-ring weight DMAs
                nc.scalar.dma_start(out=ft, in_=feats_d[b, cp])
                emit_F(b, cp, f_sb, ft)
                emit_FE_S(b, cp, f_sb, scores)

            # ---- softmax over L (single partition row)
            srow = rows.tile([1, 4], fp32, tag="srow")
            attrow = rows.tile([1, L], fp32, tag="attrow")
            if "softmax" in DEBUG_SKIP:
                nc.vector.tensor_copy(attrow, scores)
            else:
                nc.vector.tensor_reduce(srow[:, 0:1], scores,
                                        axis=mybir.AxisListType.X, op=ALU.max)
                nc.scalar.mul(srow[:, 1:2], srow[:, 0:1], -1.0)
                nc.scalar.activation(attrow, scores, AF.Exp, bias=srow[:, 1:2],
                                     scale=1.0, accum_out=srow[:, 2:3])
                nc.vector.reciprocal(srow[:, 3:4], srow[:, 2:3])
                nc.vector.tensor_scalar_mul(attrow, attrow, srow[:, 3:4])
            nc.sync.dma_start(out=att_d[b:b + 1, :], in_=attrow)

            # broadcast att across 128 partitions via PE outer product (fp32)
            attbc = apool.tile([128, L], bf16, tag="attbc")
            for c in range(4):
                ps = psmall.tile([128, 512], fp32, tag="small")
                nc.tensor.matmul(ps, ones_f, attrow[:, c * 512:(c + 1) * 512],
                                 start=True, stop=True)
                nc.vector.tensor_copy(attbc[:, c * 512:(c + 1) * 512], ps)

            # v[h] = sum_l f[h, l] * att[l]  (multiply + reduce on DVE;
            # InstTensorTensorReduce is not supported by this runtime)
            vt = vpool.tile([128, MT], fp32, tag="v")
            scr = spool.tile([128, L], bf16, tag="scr")
            if "v" in DEBUG_SKIP:
                nc.vector.memset(vt[:, :], 0.0)
            else:
                for k in range(MT):
                    nc.vector.tensor_mul(scr, f_sb[:, k, :], attbc)
                    nc.vector.tensor_reduce(vt[:, k:k + 1], scr,
                                            axis=mybir.AxisListType.X, op=ALU.add)
            # vh = v + h (bf16 for the final matmul)
            nc.vector.tensor_add(vh_bf[:, :, b], vt[:, :], h_f[:, :, b])

        # ---- out = tanh(W_out @ vh + b_out), batches as the 4-row moving dim
        out_sb = consts.tile([BC, H], fp32, tag="out_sb")
        for h2 in range(2):
            sl = slice(h2 * 512, (h2 + 1) * 512)
            ps = psmall.tile([128, 512], fp32, tag="small")
            for k in range(KT):
                nc.tensor.matmul(
                    ps[:BC, :],
                    vh_bf[:, k, :],
                    w_outT[:, k, sl],
                    start=(k == 0),
                    stop=(k == KT - 1),
                )
            nc.vector.tensor_add(out_sb[:, sl], ps[:BC, :], outbias[:, sl])
            nc.scalar.activation(out_sb[:, sl], out_sb[:, sl], AF.Tanh)
        nc.sync.dma_start(out=out_d, in_=out_sb)

    nc.compile()
    _prog_cache["nc"] = nc
    return nc


def _prep_inputs(features, hidden, W_lf, b_lf, W_ef, b_ef, W_lh, b_lh,
                 W_eh, b_eh, w_att, b_att, W_out, b_out):
    """Host-side sharding + layout prep. Returns list of 8 per-core maps."""

    def wT(W):  # [H_out, H_in] -> [128, KT, H_out] with [p,k,j] = W[j, k*128+p]
        t = np.ascontiguousarray(
            W.T.astype(BF).reshape(KT, 128, H).transpose(1, 0, 2))
        return t

    def br(v):  # [H] -> [128, MT] with [p,m] = v[m*128+p]
        return np.ascontiguousarray(
            v.astype(np.float32).reshape(MT, 128).T)

    w_lfT = wT(W_lf)
    w_efT = wT(W_ef)
    w_lhT = wT(W_lh)
    w_ehT = wT(W_eh)
    w_outT = wT(W_out)
    watt = np.ascontiguousarray(w_att.astype(BF).reshape(KT, 128).T)
    blf = br(b_lf)
    # fold b_ef into the merged bias path? b_ef enters before tanh along with
    # he; he already gets +b_eh on device. Keep b_ef separate: it is added as
    # the ACT bias of the merged tanh together with he, so fold b_ef into he
    # by adding it to b_eh's bias... he and b_ef are both per-H constants
    # added in the same place: merged = tanh(fe_mm + b_ef + he). We pass
    # bias=he via ACT; so fold b_ef into b_eh (he = W_eh@h + b_eh + b_ef).
    beh_folded = br(b_eh + b_ef)
    bef = br(b_ef)  # unused on device; kept for layout stability
    blh = br(b_lh)
    bout = np.ascontiguousarray(
        np.broadcast_to(b_out.astype(np.float32), (BC, H)))

    feats_all = features.astype(BF)
    hid_all = hidden.astype(BF)

    in_maps = []
    for c in range(NCORES):
        fs = feats_all[c * BC:(c + 1) * BC]  # [BC, E, L]
        feats_r = np.ascontiguousarray(
            fs.reshape(BC, KT, 128, 2, 1024).transpose(0, 3, 2, 1, 4))
        hs = hid_all[c * BC:(c + 1) * BC]  # [BC, H]
        hidT = np.ascontiguousarray(
            hs.T.reshape(KT, 128, BC).transpose(1, 0, 2))
        in_maps.append({
            "feats": feats_r,
            "w_lfT": w_lfT,
            "w_efT": w_efT,
            "w_lhT": w_lhT,
            "w_ehT": w_ehT,
            "w_outT": w_outT,
            "watt": watt,
            "hidT": hidT,
            "blf": blf,
            "bef": bef,
            "blh": blh,
            "beh": beh_folded,
            "bout": bout,
        })
    return in_maps


def kernel(**inputs):
    inputs = {k: np.asarray(v) for k, v in inputs.items()}
    nc = _build_program()
    in_maps = _prep_inputs(**inputs)

    from concourse.bass_utils import run_bass_kernel_spmd

    bkr = run_bass_kernel_spmd(nc, in_maps, list(range(NCORES)))
    kernel.last_results = bkr

    out = np.concatenate([bkr.results[c]["out"] for c in range(NCORES)], axis=0)
    att = np.concatenate([bkr.results[c]["att"] for c in range(NCORES)], axis=0)
    return out.astype(np.float32), att.astype(np.float32)


# revision 40
# speedup vs baseline: 150.5980x; 1.1489x over previous
"""Trainium2 Bass kernel for the AttentionLayer problem.

Computation (per batch b):
    f      = relu(W_lf @ features[b] + b_lf)         [H, L]
    fe     = W_ef @ f + b_ef                         [H, L]
    h      = tanh(W_lh @ hidden[b] + b_lh)           [H]
    he     = W_eh @ h + b_eh                         [H]
    merged = tanh(fe + he[:, None])                  [H, L]
    scores = w_att @ merged (+ b_att)                [L]
    att    = softmax(scores)                         [L]   (b_att cancels)
    v      = f @ att                                 [H]
    out    = tanh(W_out @ (v + h) + b_out)           [H]
Returns (out, att).

Strategy: data-parallel over batch B=32 across 8 NeuronCores (4 batches
per core), no collectives.  All GEMMs run on the tensor engine in bf16
with fp32 PSUM accumulation.  Weights are pre-transposed/cast on the
host so every matmul's stationary operand is a natural [K=128, M]
slice.  The v = f @ att contraction (over L, the free dim of f's
layout) runs on the vector engine as a fused multiply+reduce against a
PE-broadcast copy of att.
"""

import sys

sys.path.insert(0, "/opt/trn_rl_repo")

import numpy as np
import ml_dtypes

BF = ml_dtypes.bfloat16

B, E, H, L = 32, 1024, 1024, 2048
NCORES = 8
BC = B // NCORES  # batches per core
KT = E // 128     # contraction tiles (E == H)
MT = H // 128     # output-row tiles

_prog_cache = {}

# pool buffer tunables
PMM_BUFS = 3
PSMALL_BUFS = 2
FPOOL_BUFS = 2
FEATP_BUFS = 3
MPOOL_BUFS = 1
APOOL_BUFS = 2
SPOOL_BUFS = 1
ROWS_BUFS = 1
VPOOL_BUFS = 2
DEBUG_SKIP = set()  # build-time stage stubs for timeline attribution


def _build_program():
    """Build the single-core Bass program (identical on all 8 cores)."""
    if "nc" in _prog_cache:
        return _prog_cache["nc"]

    from contextlib import ExitStack

    import concourse.bass as bass
    import concourse.tile as tile
    from concourse import bacc, mybir

    fp32 = mybir.dt.float32
    bf16 = mybir.dt.bfloat16
    AF = mybir.ActivationFunctionType
    ALU = mybir.AluOpType
    PSUM = bass.MemorySpace.PSUM

    nc = bacc.Bacc("TRN2", target_bir_lowering=False, debug=False)

    def din(name, shape, dtype=bf16):
        return nc.declare_dram_parameter(name, list(shape), dtype, isOutput=False).ap()

    # [b, cp, p, k, n] = features[b, k*128+p, cp*1024+n], bf16
    feats_d = din("feats", [BC, 2, 128, KT, 1024])
    w_lfT_d = din("w_lfT", [128, KT, H])
    w_efT_d = din("w_efT", [128, KT, H])
    w_lhT_d = din("w_lhT", [128, KT, H])
    w_ehT_d = din("w_ehT", [128, KT, H])
    w_outT_d = din("w_outT", [128, KT, H])
    watt_d = din("watt", [128, KT])
    hidT_d = din("hidT", [128, KT, BC])
    blf_d = din("blf", [128, MT], fp32)
    bef_d = din("bef", [128, MT], fp32)
    blh_d = din("blh", [128, MT], fp32)
    beh_d = din("beh", [128, MT], fp32)
    bout_d = din("bout", [BC, H], fp32)

    out_d = nc.declare_dram_parameter("out", [BC, H], fp32, isOutput=True).ap()
    att_d = nc.declare_dram_parameter("att", [BC, L], fp32, isOutput=True).ap()

    with ExitStack() as ctx:
        tc = ctx.enter_context(tile.TileContext(nc))

        # PSUM: pmm 3x[128,1024] (6 banks) + psmall 2x[<=128,512] (2 banks)
        pmm = ctx.enter_context(tc.tile_pool(name="pmm", bufs=PMM_BUFS, space=PSUM))
        psmall = ctx.enter_context(
            tc.tile_pool(name="psmall", bufs=PSMALL_BUFS, space=PSUM))

        consts = ctx.enter_context(tc.tile_pool(name="consts", bufs=1))

        def cload(dram_ap, shape, dtype, tag):
            # small constants ride the gpsimd SWDGE ring, leaving the sync
            # HWDGE ring free for the big weight transfers
            t = consts.tile(list(shape), dtype, tag=tag)
            nc.gpsimd.dma_start(out=t, in_=dram_ap)
            return t

        # hidT gates the very first PE matmul: sync ring, ahead of weights
        hidT = consts.tile([128, KT, BC], bf16, tag="hidT")
        nc.sync.dma_start(out=hidT, in_=hidT_d)
        blh = cload(blh_d, [128, MT], fp32, "blh")
        watt = cload(watt_d, [128, KT], bf16, "watt")
        blf = cload(blf_d, [128, MT], fp32, "blf")
        beh = cload(beh_d, [128, MT], fp32, "beh")
        outbias = cload(bout_d, [BC, H], fp32, "outbias")

        ones_f = consts.tile([1, 128], fp32, tag="ones")
        nc.vector.memset(ones_f[:, :], 1.0)
        ones_b = consts.tile([1, 128], bf16, tag="onesb")
        nc.vector.memset(ones_b[:, :], 1.0)

        h_f = consts.tile([128, MT, BC], fp32, tag="h_f")
        h_bf = consts.tile([128, MT, BC], bf16, tag="h_bf")
        he_f = consts.tile([128, MT, BC], fp32, tag="he_f")
        vh_bf = consts.tile([128, MT, BC], bf16, tag="vh_bf")

        # k-halved weight DMAs: PE consumes halves as they arrive instead of
        # stalling on one 2MB transfer; order matches PE stream consumption
        w_lfT = consts.tile([128, KT, H], bf16, tag="w_lfT")
        for k in range(KT):
            nc.sync.dma_start(out=w_lfT[:, k, :], in_=w_lfT_d[:, k, :])
        w_efT = consts.tile([128, KT, H], bf16, tag="w_efT")
        for k in range(0, KT, 4):
            nc.sync.dma_start(out=w_efT[:, k:k + 4, :],
                              in_=w_efT_d[:, k:k + 4, :])

        # ---- main pools
        fpool = ctx.enter_context(tc.tile_pool(name="fpool", bufs=FPOOL_BUFS))
        featp = ctx.enter_context(tc.tile_pool(name="featp", bufs=FEATP_BUFS))
        mpool = ctx.enter_context(tc.tile_pool(name="mpool", bufs=MPOOL_BUFS))
        apool = ctx.enter_context(tc.tile_pool(name="apool", bufs=APOOL_BUFS))
        spool = ctx.enter_context(tc.tile_pool(name="spool", bufs=SPOOL_BUFS))
        rows = ctx.enter_context(tc.tile_pool(name="rows", bufs=ROWS_BUFS))
        vpool = ctx.enter_context(tc.tile_pool(name="vpool", bufs=VPOOL_BUFS))

        def emit_hidden():
            # hidden path rides behind F(b0,cp0) on the PE stream; its
            # weights stream through the featp pool (same tile shape)
            w_lhT = featp.tile([128, KT, H], bf16, tag="ft")
            nc.sync.dma_start(out=w_lhT, in_=w_lhT_d)
            w_ehT = featp.tile([128, KT, H], bf16, tag="ft")
            nc.sync.dma_start(out=w_ehT, in_=w_ehT_d)
            if "hidden" in DEBUG_SKIP:
                nc.vector.memset(h_f[:, :, :], 0.0)
                nc.vector.memset(h_bf[:, :, :], 0.0)
                nc.vector.memset(he_f[:, :, :], 0.0)
            for m in range(MT) if "hidden" not in DEBUG_SKIP else []:
                ps = psmall.tile([128, 512], fp32, tag="small")
                for k in range(KT):
                    nc.tensor.matmul(
                        ps[:, :BC],
                        w_lhT[:, k, m * 128:(m + 1) * 128],
                        hidT[:, k, :],
                        start=(k == 0),
                        stop=(k == KT - 1),
                    )
                nc.scalar.activation(h_f[:, m, :], ps[:, :BC], AF.Tanh,
                                     bias=blh[:, m:m + 1])
            if "hidden" not in DEBUG_SKIP:
                nc.vector.tensor_copy(h_bf[:, :, :], h_f[:, :, :])
            for m in range(MT) if "hidden" not in DEBUG_SKIP else []:
                ps = psmall.tile([128, 512], fp32, tag="small")
                for k in range(KT):
                    nc.tensor.matmul(
                        ps[:, :BC],
                        w_ehT[:, k, m * 128:(m + 1) * 128],
                        h_bf[:, k, :],
                        start=(k == 0),
                        stop=(k == KT - 1),
                    )
                nc.scalar.activation(he_f[:, m, :], ps[:, :BC], AF.Identity,
                                     bias=beh[:, m:m + 1])

        def emit_F(b, cp, f_sb, ft):
            # f = relu(W_lf @ features + b_lf), cast to bf16
            for m in range(MT):
                ps = pmm.tile([128, 1024], fp32, tag="mm")
                for h2 in range(2):
                    sl = slice(h2 * 512, (h2 + 1) * 512)
                    for k in range(KT):
                        nc.tensor.matmul(
                            ps[:, sl],
                            w_lfT[:, k, m * 128:(m + 1) * 128],
                            ft[:, k, sl],
                            start=(k == 0),
                            stop=(k == KT - 1),
                        )
                nc.scalar.activation(
                    f_sb[:, m, cp * 1024:(cp + 1) * 1024], ps, AF.Relu,
                    bias=blf[:, m:m + 1])

        def emit_FE_S(b, cp, f_sb, exprow, srow):
            # merged = tanh(W_ef @ f + (he + b_eh + b_ef))
            merged = mpool.tile([128, MT, 1024], bf16, tag="m")
            for m in range(MT):
                ps = pmm.tile([128, 1024], fp32, tag="mm")
                for h2 in range(2):
                    sl = slice(h2 * 512, (h2 + 1) * 512)
                    for k in range(KT):
                        nc.tensor.matmul(
                            ps[:, sl],
                            w_efT[:, k, m * 128:(m + 1) * 128],
                            f_sb[:, k, cp * 1024 + h2 * 512:
                                 cp * 1024 + (h2 + 1) * 512],
                            start=(k == 0),
                            stop=(k == KT - 1),
                        )
                nc.scalar.activation(merged[:, m, :], ps, AF.Tanh,
                                     bias=he_f[:, m, b:b + 1])
            # scores = w_att @ merged; exp immediately (no max subtraction:
            # |scores| <= ||w_att||_1 * 1 stays far from fp32 exp overflow),
            # accumulating the partial sum of exps per 512-column half
            for h2 in range(2):
                ps = psmall.tile([128, 512], fp32, tag="small")
                for k in range(KT):
                    nc.tensor.matmul(
                        ps[:1, :],
                        watt[:, k:k + 1],
                        merged[:, k, h2 * 512:(h2 + 1) * 512],
                        start=(k == 0),
                        stop=(k == KT - 1),
                    )
                part = cp * 2 + h2
                nc.scalar.activation(
                    exprow[:, cp * 1024 + h2 * 512:cp * 1024 + (h2 + 1) * 512],
                    ps[:1, :], AF.Exp, accum_out=srow[:, part:part + 1])

        def emit_V(b, cp, f_sb, exprow, expbf, vt):
            # unnormalized v += f[:, chunk] @ exp(scores[chunk])
            attbc = apool.tile([128, 1024], bf16, tag="attbc")
            nc.vector.tensor_copy(expbf[:, cp * 1024:(cp + 1) * 1024],
                                  exprow[:, cp * 1024:(cp + 1) * 1024])
            for h2 in range(2):
                ps = psmall.tile([128, 512], fp32, tag="small")
                nc.tensor.matmul(
                    ps, ones_b,
                    expbf[:, cp * 1024 + h2 * 512:cp * 1024 + (h2 + 1) * 512],
                    start=True, stop=True)
                nc.vector.tensor_copy(attbc[:, h2 * 512:(h2 + 1) * 512], ps)
            scr = spool.tile([128, 1024], bf16, tag="scr")
            for k in range(MT):
                # fused multiply + free-dim accumulate in one DVE op
                if cp == 0:
                    nc.vector.scalar_tensor_tensor(
                        out=scr, in0=f_sb[:, k, cp * 1024:(cp + 1) * 1024],
                        scalar=1.0, in1=attbc, op0=ALU.mult, op1=ALU.mult,
                        accum_out=vt[:, k:k + 1])
                else:
                    vtmp = vpool.tile([128, 1], fp32, tag="vtmp")
                    nc.vector.scalar_tensor_tensor(
                        out=scr, in0=f_sb[:, k, cp * 1024:(cp + 1) * 1024],
                        scalar=1.0, in1=attbc, op0=ALU.mult, op1=ALU.mult,
                        accum_out=vtmp)
                    nc.vector.tensor_add(vt[:, k:k + 1], vt[:, k:k + 1], vtmp)

        for b in range(BC):
            f_sb = fpool.tile([128, KT, L], bf16, tag="f")
            exprow = rows.tile([1, L], fp32, tag="exprow")
            expbf = rows.tile([1, L], bf16, tag="expbf")
            srow = rows.tile([1, 8], fp32, tag="srow")
            vt = vpool.tile([128, MT], fp32, tag="v")

            if b == BC - 1:
                # prefetch the final-GEMM weights while b3 computes
                w_outT = featp.tile([128, KT, H], bf16, tag="ft")
                nc.sync.dma_start(out=w_outT, in_=w_outT_d)
            for cp in range(2):
                ft = featp.tile([128, KT, 1024], bf16, tag="ft")
                # scalar HWDGE ring: runs parallel to the sync-ring weight DMAs
                if b == 0 and cp == 0:
                    # k-sliced so the first F matmul isn't gated on 2MB
                    for k in range(KT):
                        nc.scalar.dma_start(out=ft[:, k, :],
                                            in_=feats_d[b, cp, :, k, :])
                else:
                    nc.scalar.dma_start(out=ft, in_=feats_d[b, cp])
                emit_F(b, cp, f_sb, ft)
                if b == 0 and cp == 0:
                    emit_hidden()
                emit_FE_S(b, cp, f_sb, exprow, srow)
                if cp == 1:
                    # Z / reciprocal / partition-broadcast as soon as all 4
                    # partial exp-sums exist, so only the tensor_scalar_mul
                    # and vh add trail the final v chunk
                    nc.vector.tensor_reduce(srow[:, 4:5], srow[:, 0:4],
                                            axis=mybir.AxisListType.X,
                                            op=ALU.add)
                    nc.vector.reciprocal(srow[:, 5:6], srow[:, 4:5])
                    ps = psmall.tile([128, 512], fp32, tag="small")
                    nc.tensor.matmul(ps[:, :1], ones_f, srow[:, 5:6],
                                     start=True, stop=True)
                    rcpb = vpool.tile([128, 1], fp32, tag="rcpb")
                    nc.vector.tensor_copy(rcpb, ps[:, :1])
                if "v" not in DEBUG_SKIP and not (b == BC - 1 and cp == 1):
                    emit_V(b, cp, f_sb, exprow, expbf, vt)

            if b < BC - 1:
                if "v" in DEBUG_SKIP:
                    nc.vector.memset(vt[:, :], 0.0)
                nc.vector.tensor_scalar_mul(vt[:, :], vt[:, :], rcpb)
                # vh = v + h (bf16 for the final matmul)
                nc.vector.tensor_add(vh_bf[:, :, b], vt[:, :], h_f[:, :, b])
            else:
                # last batch: fuse the final v chunk, normalization, and the
                # out-GEMM per k-tile so PE consumes each vh slice as soon as
                # the DVE produces it (shortens the end-of-kernel tail)
                attbc = apool.tile([128, 1024], bf16, tag="attbc")
                nc.vector.tensor_copy(expbf[:, 1024:], exprow[:, 1024:])
                for h2 in range(2):
                    ps = psmall.tile([128, 512], fp32, tag="small")
                    nc.tensor.matmul(
                        ps, ones_b, expbf[:, 1024 + h2 * 512:1024 + (h2 + 1) * 512],
                        start=True, stop=True)
                    nc.vector.tensor_copy(attbc[:, h2 * 512:(h2 + 1) * 512], ps)
                out_big = pmm.tile([128, 1024], fp32, tag="mm")
                out_ps = [out_big[:, 0:512], out_big[:, 512:1024]]
                scr = spool.tile([128, 1024], bf16, tag="scr")
                for k in range(MT):
                    if "v" in DEBUG_SKIP:
                        nc.vector.memset(vt[:, k:k + 1], 0.0)
                    else:
                        vtmp = vpool.tile([128, 1], fp32, tag="vtmp")
                        nc.vector.scalar_tensor_tensor(
                            out=scr, in0=f_sb[:, k, 1024:], scalar=1.0,
                            in1=attbc, op0=ALU.mult, op1=ALU.mult,
                            accum_out=vtmp)
                        nc.vector.tensor_add(vt[:, k:k + 1], vt[:, k:k + 1],
                                             vtmp)
                    nc.vector.tensor_scalar_mul(vt[:, k:k + 1], vt[:, k:k + 1],
                                                rcpb)
                    nc.vector.tensor_add(vh_bf[:, k, b:b + 1], vt[:, k:k + 1],
                                         h_f[:, k, b:b + 1])
                    for h2 in range(2):
                        nc.tensor.matmul(
                            out_ps[h2][:BC, :],
                            vh_bf[:, k, :],
                            w_outT[:, k, h2 * 512:(h2 + 1) * 512],
                            start=(k == 0),
                            stop=(k == KT - 1),
                        )

            # att output: normalize exp in place and DMA (off critical path)
            nc.scalar.mul(exprow, exprow, srow[:, 5:6])
            nc.sync.dma_start(out=att_d[b:b + 1, :], in_=exprow)

        # ---- out = tanh(W_out @ vh + b_out) epilogue
        out_sb = consts.tile([BC, H], fp32, tag="out_sb")
        for h2 in range(2):
            sl = slice(h2 * 512, (h2 + 1) * 512)
            nc.vector.tensor_add(out_sb[:, sl], out_ps[h2][:BC, :],
                                 outbias[:, sl])
            nc.scalar.activation(out_sb[:, sl], out_sb[:, sl], AF.Tanh)
        nc.sync.dma_start(out=out_d, in_=out_sb)

    nc.compile()
    _prog_cache["nc"] = nc
    return nc


def _prep_inputs(features, hidden, W_lf, b_lf, W_ef, b_ef, W_lh, b_lh,
                 W_eh, b_eh, w_att, b_att, W_out, b_out):
    """Host-side sharding + layout prep. Returns list of 8 per-core maps."""

    def wT(W):  # [H_out, H_in] -> [128, KT, H_out] with [p,k,j] = W[j, k*128+p]
        t = np.ascontiguousarray(
            W.T.astype(BF).reshape(KT, 128, H).transpose(1, 0, 2))
        return t

    def br(v):  # [H] -> [128, MT] with [p,m] = v[m*128+p]
        return np.ascontiguousarray(
            v.astype(np.float32).reshape(MT, 128).T)

    w_lfT = wT(W_lf)
    w_efT = wT(W_ef)
    w_lhT = wT(W_lh)
    w_ehT = wT(W_eh)
    w_outT = wT(W_out)
    watt = np.ascontiguousarray(w_att.astype(BF).reshape(KT, 128).T)
    blf = br(b_lf)
    # fold b_ef into the merged bias path? b_ef enters before tanh along with
    # he; he already gets +b_eh on device. Keep b_ef separate: it is added as
    # the ACT bias of the merged tanh together with he, so fold b_ef into he
    # by adding it to b_eh's bias... he and b_ef are both per-H constants
    # added in the same place: merged = tanh(fe_mm + b_ef + he). We pass
    # bias=he via ACT; so fold b_ef into b_eh (he = W_eh@h + b_eh + b_ef).
    beh_folded = br(b_eh + b_ef)
    bef = br(b_ef)  # unused on device; kept for layout stability
    blh = br(b_lh)
    bout = np.ascontiguousarray(
        np.broadcast_to(b_out.astype(np.float32), (BC, H)))

    feats_all = features.astype(BF)
    hid_all = hidden.astype(BF)

    in_maps = []
    for c in range(NCORES):
        fs = feats_all[c * BC:(c + 1) * BC]  # [BC, E, L]
        feats_r = np.ascontiguousarray(
            fs.reshape(BC, KT, 128, 2, 1024).transpose(0, 3, 2, 1, 4))
        hs = hid_all[c * BC:(c + 1) * BC]  # [BC, H]
        hidT = np.ascontiguousarray(
            hs.T.reshape(KT, 128, BC).transpose(1, 0, 2))
        in_maps.append({
            "feats": feats_r,
            "w_lfT": w_lfT,
            "w_efT": w_efT,
            "w_lhT": w_lhT,
            "w_ehT": w_ehT,
            "w_outT": w_outT,
            "watt": watt,
            "hidT": hidT,
            "blf": blf,
            "bef": bef,
            "blh": blh,
            "beh": beh_folded,
            "bout": bout,
        })
    return in_maps


def kernel(**inputs):
    inputs = {k: np.asarray(v) for k, v in inputs.items()}
    nc = _build_program()
    in_maps = _prep_inputs(**inputs)

    from concourse.bass_utils import run_bass_kernel_spmd

    bkr = run_bass_kernel_spmd(nc, in_maps, list(range(NCORES)))
    kernel.last_results = bkr

    out = np.concatenate([bkr.results[c]["out"] for c in range(NCORES)], axis=0)
    att = np.concatenate([bkr.results[c]["att"] for c in range(NCORES)], axis=0)
    return out.astype(np.float32), att.astype(np.float32)
